# revision 4
# baseline (speedup 1.0000x reference)
"""Trainium2 Bass kernel for nn_Inv1x1ConvPermute.

out[b,t,o] = sum_i x[b,t,i] * kernel[i,o]   (kernel is a CxC permutation matrix)

Strategy: pure data parallel over 8 NeuronCores — core i takes 2 of the 16
batches (32768 tokens of 256 channels). On-device per 128-token subtile:
  1. DMA x tile [128 tok, 256 ch] (token-major, contiguous rows)
  2. PE transpose (fp32) -> xT chunks [128 ch, 128 tok] in PSUM
  3. DVE copy PSUM -> SBUF
  4. PE matmul out[tok, :] = xT.T @ kernel  (fp32, accumulate over 2 K-chunks)
  5. ACT copy PSUM -> SBUF, DMA out
The permutation matmul in fp32 is bit-exact (products are x*1.0 or x*0.0).
"""

import os

import numpy as np

import concourse.bacc as bacc
import concourse.mybir as mybir
import concourse.tile as tile
from concourse.bass_utils import run_bass_kernel_spmd

B, T, C = 16, 16384, 256
N_CORES = 8
P = 128
TOK_PER_CORE = B * T // N_CORES  # 32768


def build_nc(n_tok: int, sub: int = 4):
    """Build + compile the per-core Bass program for n_tok tokens."""
    nc = bacc.Bacc(
        "TRN2", target_bir_lowering=False, debug=False, num_devices=N_CORES
    )
    f32 = mybir.dt.float32
    x = nc.dram_tensor("x", [n_tok, C], f32, kind="ExternalInput").ap()
    kmat = nc.dram_tensor("kmat", [C, C], f32, kind="ExternalInput").ap()
    ident = nc.dram_tensor("ident", [P, P], f32, kind="ExternalInput").ap()
    out = nc.dram_tensor("out", [n_tok, C], f32, kind="ExternalOutput").ap()

    blk_tok = P * sub
    nblk = n_tok // blk_tok
    assert n_tok % blk_tok == 0

    with tile.TileContext(nc) as tc:
        with (
            tc.tile_pool(name="const", bufs=1) as cpool,
            tc.tile_pool(name="xin", bufs=3) as xpool,
            tc.tile_pool(name="xt", bufs=4) as xtpool,
            tc.tile_pool(name="outp", bufs=3) as opool,
            tc.tile_pool(name="pst", bufs=3, space="PSUM") as pst,
            tc.tile_pool(name="pso", bufs=3, space="PSUM") as pso,
        ):
            k_sb = cpool.tile([P, 2 * C], f32)
            for kc in range(2):
                nc.sync.dma_start(
                    out=k_sb[:, kc * C : (kc + 1) * C],
                    in_=kmat[kc * P : (kc + 1) * P, :],
                )
            id_sb = cpool.tile([P, P], f32)
            nc.sync.dma_start(out=id_sb[:], in_=ident)

            for b in range(nblk):
                xt_in = xpool.tile([P, sub * C], f32)
                src = x[b * blk_tok : (b + 1) * blk_tok, :].rearrange(
                    "(j p) c -> p j c", p=P
                )
                nc.sync.dma_start(
                    out=xt_in[:].rearrange("p (j c) -> p j c", j=sub), in_=src
                )
                out_sb = opool.tile([P, sub * C], f32)
                for j in range(sub):
                    outp = pso.tile([P, C], f32)
                    for kc in range(2):
                        pT = pst.tile([P, P], f32)
                        nc.tensor.transpose(
                            pT[:],
                            xt_in[:, j * C + kc * P : j * C + (kc + 1) * P],
                            id_sb[:],
                        )
                        xT = xtpool.tile([P, P], f32)
                        nc.vector.tensor_copy(xT[:], pT[:])
                        nc.tensor.matmul(
                            outp[:],
                            xT[:],
                            k_sb[:, kc * C : (kc + 1) * C],
                            start=(kc == 0),
                            stop=(kc == 1),
                        )
                    nc.scalar.copy(out_sb[:, j * C : (j + 1) * C], outp[:])
                dst = out[b * blk_tok : (b + 1) * blk_tok, :].rearrange(
                    "(j p) c -> p j c", p=P
                )
                nc.sync.dma_start(
                    out=dst, in_=out_sb[:].rearrange("p (j c) -> p j c", j=sub)
                )
    nc.compile()
    return nc


_LAST_RESULT = {}


def kernel(x, kernel):
    x = np.ascontiguousarray(np.asarray(x, dtype=np.float32))
    kmat = np.ascontiguousarray(np.asarray(kernel, dtype=np.float32))
    assert x.shape == (B, T, C) and kmat.shape == (C, C)

    xs = x.reshape(N_CORES, TOK_PER_CORE, C)
    ident = np.eye(P, dtype=np.float32)
    in_maps = [
        {"x": xs[i], "kmat": kmat, "ident": ident} for i in range(N_CORES)
    ]

    nc = build_nc(TOK_PER_CORE)
    res = run_bass_kernel_spmd(nc, in_maps, list(range(N_CORES)))
    _LAST_RESULT["res"] = res
    if res.exec_time_ns is not None:
        print(f"HW exec time: {res.exec_time_ns} ns")

    outs = [res.results[i]["out"] for i in range(N_CORES)]
    full = np.stack(outs, axis=0).reshape(B, T, C).astype(np.float32)
    return full


# revision 7
# speedup vs baseline: 1.0492x; 1.0492x over previous
"""Trainium2 Bass kernel for nn_Inv1x1ConvPermute.

out[b,t,o] = sum_i x[b,t,i] * kernel[i,o]   (kernel is a CxC permutation matrix)

Strategy: pure data parallel over 8 NeuronCores — core i takes 2 of the 16
batches (32768 tokens of 256 channels). On-device per 128-token subtile:
  1. DMA x tile [128 tok, 256 ch] (token-major, contiguous rows)
  2. PE transpose (fp32) -> xT chunks [128 ch, 128 tok] in PSUM
  3. DVE copy PSUM -> SBUF
  4. PE matmul out[tok, :] = xT.T @ kernel  (fp32, accumulate over 2 K-chunks)
  5. ACT copy PSUM -> SBUF, DMA out
The permutation matmul in fp32 is bit-exact (products are x*1.0 or x*0.0).
"""

import os

import numpy as np

import concourse.bacc as bacc
import concourse.mybir as mybir
import concourse.tile as tile
from concourse.bass_utils import run_bass_kernel_spmd

B, T, C = 16, 16384, 256
N_CORES = 8
P = 128
TOK_PER_CORE = B * T // N_CORES  # 32768


def build_nc(n_tok: int, sub: int = 8):
    """Build + compile the per-core Bass program for n_tok tokens."""
    nc = bacc.Bacc(
        "TRN2", target_bir_lowering=False, debug=False, num_devices=N_CORES
    )
    f32 = mybir.dt.float32
    x = nc.dram_tensor("x", [n_tok, C], f32, kind="ExternalInput").ap()
    kmat = nc.dram_tensor("kmat", [C, C], f32, kind="ExternalInput").ap()
    ident = nc.dram_tensor("ident", [P, P], f32, kind="ExternalInput").ap()
    out = nc.dram_tensor("out", [n_tok, C], f32, kind="ExternalOutput").ap()

    blk_tok = P * sub
    nblk = n_tok // blk_tok
    assert n_tok % blk_tok == 0

    with tile.TileContext(nc) as tc:
        with (
            tc.tile_pool(name="const", bufs=1) as cpool,
            tc.tile_pool(name="xin", bufs=3) as xpool,
            tc.tile_pool(name="xt", bufs=6) as xtpool,
            tc.tile_pool(name="outp", bufs=3) as opool,
            tc.tile_pool(name="pst", bufs=4, space="PSUM") as pst,
            tc.tile_pool(name="pso", bufs=3, space="PSUM") as pso,
        ):
            k_sb = cpool.tile([P, 2 * C], f32)
            for kc in range(2):
                nc.sync.dma_start(
                    out=k_sb[:, kc * C : (kc + 1) * C],
                    in_=kmat[kc * P : (kc + 1) * P, :],
                )
            id_sb = cpool.tile([P, P], f32)
            nc.sync.dma_start(out=id_sb[:], in_=ident)

            for b in range(nblk):
                xt_in = xpool.tile([P, sub * C], f32)
                src = x[b * blk_tok : (b + 1) * blk_tok, :].rearrange(
                    "(j p) c -> p j c", p=P
                )
                nc.sync.dma_start(
                    out=xt_in[:].rearrange("p (j c) -> p j c", j=sub), in_=src
                )
                out_sb = opool.tile([P, sub * C], f32)
                for j in range(sub):
                    outp = pso.tile([P, C], f32)
                    for kc in range(2):
                        pT = pst.tile([P, P], f32)
                        nc.tensor.transpose(
                            pT[:],
                            xt_in[:, j * C + kc * P : j * C + (kc + 1) * P],
                            id_sb[:],
                        )
                        xT = xtpool.tile([P, P], f32)
                        # balance PSUM->SBUF traffic across DVE and ACT
                        if kc == 0:
                            nc.vector.tensor_copy(xT[:], pT[:])
                        else:
                            nc.scalar.copy(xT[:], pT[:])
                        nc.tensor.matmul(
                            outp[:],
                            xT[:],
                            k_sb[:, kc * C : (kc + 1) * C],
                            start=(kc == 0),
                            stop=(kc == 1),
                        )
                    if j % 2 == 0:
                        nc.scalar.copy(out_sb[:, j * C : (j + 1) * C], outp[:])
                    else:
                        nc.vector.tensor_copy(out_sb[:, j * C : (j + 1) * C], outp[:])
                dst = out[b * blk_tok : (b + 1) * blk_tok, :].rearrange(
                    "(j p) c -> p j c", p=P
                )
                # stores on the ACT HWDGE ring, loads on the SP ring
                nc.scalar.dma_start(
                    out=dst, in_=out_sb[:].rearrange("p (j c) -> p j c", j=sub)
                )
    nc.compile()
    return nc


_LAST_RESULT = {}


def kernel(x, kernel):
    x = np.ascontiguousarray(np.asarray(x, dtype=np.float32))
    kmat = np.ascontiguousarray(np.asarray(kernel, dtype=np.float32))
    assert x.shape == (B, T, C) and kmat.shape == (C, C)

    xs = x.reshape(N_CORES, TOK_PER_CORE, C)
    ident = np.eye(P, dtype=np.float32)
    in_maps = [
        {"x": xs[i], "kmat": kmat, "ident": ident} for i in range(N_CORES)
    ]

    nc = build_nc(TOK_PER_CORE)
    res = run_bass_kernel_spmd(nc, in_maps, list(range(N_CORES)))
    _LAST_RESULT["res"] = res
    if res.exec_time_ns is not None:
        print(f"HW exec time: {res.exec_time_ns} ns")

    outs = [res.results[i]["out"] for i in range(N_CORES)]
    full = np.stack(outs, axis=0).reshape(B, T, C).astype(np.float32)
    return full
